# revision 49
# baseline (speedup 1.0000x reference)
"""Causal self-attention (B=4, T=2048, C=1024, NH=16) on 8 TRN2 NeuronCores.

Sharding: core c -> batch b = c//2, head-group g = c%2 (8 heads, D=512).
Each core computes q/k/v projections for its head group on its batch,
fused causal attention (attT layout: k on partitions), and a partial
output projection through its row-slice of Wp. Host sums the two
partials per batch.

All operands bf16 (host pre-casts x/W): halves input DMA and enables
the compiler's automatic Fast Weight Load (FWL needs non-fp32 128-col
weights), so LDWEIGHTS overlaps the matmul stream.

Single fused loop over q-blocks jq (512 q each):
  proj(jq): qt/kt m-chunks into one [128,1024] psum (q|k halves, one
  DVE copy), v chunks scattered into per-k-chunk AV lhsT slots whose
  per-head ones column makes softmax denominators a free psum row.
  norm+outproj(jq-1): deferred one block so the PE never waits on the
  ACT recip chain (s rows live on partitions {hp} / {64+hp}; Ln+Exp on
  [4,512] tiles; PE bcm matmul broadcasts 1/s; one TT mul per (hp)).
  attn(jq): per head pair, QK chunk pair into [128,1024] psum, ONE exp
  per chunk ([2,512-off] strided AP), tri-mask on diagonal chunks,
  software-pipelined AV that streams only the valid [off:512] columns.

kernel(**inputs) takes the FULL unsharded inputs and returns the FULL
output. Self-contained: hardcodes all shapes, reads nothing from disk.
"""

import sys

sys.path.insert(0, "/opt/trn_rl_repo")

import numpy as np
import ml_dtypes
from contextlib import ExitStack

import concourse.bass as bass  # noqa: F401
import concourse.mybir as mybir
import concourse.tile as tile
from concourse import bacc
from concourse.bass_utils import run_bass_kernel_spmd

P = 128
B, T, C = 4, 2048, 1024
NH, HS = 16, 64
D = 512          # per-core head dim (8 heads)
H = 8            # local heads
NCO = C // P     # 8 contraction chunks
NKC = T // P     # 16 k chunks
NJQ = T // 512   # 4 q blocks
f32 = mybir.dt.float32
bf16 = mybir.dt.bfloat16
AFT = mybir.ActivationFunctionType


def g2(ap):
    """View a [P, 1024] AP as [P, 2, 512] (even|odd halves)."""
    return ap.rearrange("p (g c) -> p g c", g=2)


def build_nc():
    nc = bacc.Bacc("TRN2", target_bir_lowering=False, debug=False, num_devices=8)

    # host pre-shuffled layouts: per-partition bytes contiguous (4-8KB
    # DMA descriptors instead of 1KB rows)
    xt_d = nc.dram_tensor("xt", [NJQ, P, NCO, 512], bf16, kind="ExternalInput")
    wq_d = nc.dram_tensor("wq", [4, P, NCO, P], bf16, kind="ExternalInput")
    wk_d = nc.dram_tensor("wk", [4, P, NCO, P], bf16, kind="ExternalInput")
    wv_d = nc.dram_tensor("wv", [P, NCO, D], bf16, kind="ExternalInput")
    wp_d = nc.dram_tensor("wp", [D, C], bf16, kind="ExternalInput")
    tri_d = nc.dram_tensor("tri", [P, 2, P], bf16, kind="ExternalInput")
    bcm_d = nc.dram_tensor("bcm", [8, 4, P], bf16, kind="ExternalInput")
    out_d = nc.dram_tensor("out", [T, C], bf16, kind="ExternalOutput")

    wp_r = wp_d[:].rearrange("(dc p) c -> p dc c", p=P)
    out_r = out_d[:].rearrange("(tc p) c -> p tc c", p=P)

    with tile.TileContext(nc) as tc, ExitStack() as ctx, nc.allow_low_precision(
        reason="bf16 attention kernel"
    ):
        perm = ctx.enter_context(tc.tile_pool(name="perm", bufs=1))
        work = ctx.enter_context(tc.tile_pool(name="work", bufs=1))
        psum = ctx.enter_context(tc.tile_pool(name="psum", bufs=2, space="PSUM"))

        wq_sb = perm.tile([P, 4, NCO, P], bf16)  # [p, m, co, dd]
        qkt_sb = perm.tile([P, 2, 4, T], bf16)   # [dh-pair, q|k, hp, t]
        wk_sb = perm.tile([P, 4, NCO, P], bf16)
        wv_sb = perm.tile([P, NCO, D], bf16)
        wp_sb = perm.tile([P, 4, C], bf16)
        v_sb = perm.tile([P, NKC, H, P], bf16)   # per-chunk AV lhsT slots
        yt_sb = perm.tile([P, 4, T], bf16)
        s2_sb = perm.tile([P, 4, 512], f32)      # raw s rows 0 (odd) / 64 (even)
        sc_sb = perm.tile([8, NJQ, 512], f32)    # gathered s, 8 partitions
        scr_sb = perm.tile([8, NJQ, 512], f32)   # 1/s
        scb_sb = perm.tile([8, NJQ, 512], bf16)  # 1/s as bf16 matmul operand
        tri_sb = perm.tile([P, 2, P], bf16)
        bcm_sb = perm.tile([8, 4, P], bf16)

        # earliest-needed first: wq m0 + first x co-chunk gate the first
        # matmul; the rest is emitted inside the loop, most-urgent first
        nc.sync.dma_start(wq_sb[:, 0], wq_d[0])

        dum_sb = perm.tile([P, 512], bf16)
        nc.gpsimd.memset(dum_sb[:], 0.0)
        # warm the PE p-state during the DMA-init window: ~40 back-to-back
        # dummy matmuls ramp the clock to 2.4GHz before real data lands
        for i in range(30):
            pd = psum.tile([P, 512], f32, tag=("pe" if i % 2 == 0 else "po"),
                           bufs=1, name=f"dum{i}")
            nc.tensor.matmul(
                pd[:], dum_sb[:, 0:128], dum_sb[:], start=True, stop=True
            )

        nc.gpsimd.memset(v_sb[:], 0.0)
        nc.gpsimd.memset(sc_sb[:], 1.0)
        nc.gpsimd.memset(scb_sb[:], 1.0)
        v5 = v_sb[:].rearrange("p k (hp par) c -> p k hp par c", par=2)
        # ones cols: even head -> col 64 (s at psum row 64); odd -> col 0
        nc.gpsimd.memset(v5[:, :, :, 0, 64:65], 1.0)
        nc.gpsimd.memset(v5[:, :, :, 1, 0:1], 1.0)

        def emit_recip(jq):
            nc.vector.reciprocal_approx_fast(
                out=scr_sb[0:8, jq, :], in_=sc_sb[0:8, jq, :]
            )
            nc.vector.tensor_copy(out=scb_sb[0:8, jq, :], in_=scr_sb[0:8, jq, :])

        def emit_norm(jq):
            blk = slice(jq * 512, (jq + 1) * 512)
            emit_recip(jq)
            for pair in range(2):
                rb2 = psum.tile([P, 1024], f32, tag="qk2", bufs=3)
                for half in range(2):
                    hp = 2 * pair + half
                    nc.tensor.matmul(
                        rb2[:, half * 512 : (half + 1) * 512],
                        bcm_sb[0:8, hp, :], scb_sb[0:8, jq, :],
                        start=True, stop=True,
                    )
                for half in range(2):
                    hp = 2 * pair + half
                    nc.vector.tensor_mul(
                        out=yt_sb[:, hp, blk], in0=yt_sb[:, hp, blk],
                        in1=g2(rb2[:])[:, half, :],
                    )

        def emit_norm_hp(jq, hp):
            blk = slice(jq * 512, (jq + 1) * 512)
            rbh = psum.tile([P, 1024], f32, tag="qk2", bufs=3)
            nc.tensor.matmul(
                rbh[:, 0:512], bcm_sb[0:8, hp, :], scb_sb[0:8, jq, :],
                start=True, stop=True,
            )
            nc.vector.tensor_mul(
                out=yt_sb[:, hp, blk], in0=yt_sb[:, hp, blk],
                in1=rbh[:, 0:512],
            )

        def emit_outproj_tcn(jq, t4, use_act=False):
            tcn = jq * 4 + t4
            po2 = psum.tile([P, 1024], f32, tag="qk2", bufs=3)
            ob = work.tile([P, C], bf16, tag="ob", bufs=2)
            # per-half psum->sbuf copies, emitted right after each half's
            # last matmul: frees the psum buffer sooner (the next QK pair
            # WARs this tile) and overlaps copy with the other half
            for half in range(2):
                hs = slice(half * 512, (half + 1) * 512)
                for dc in range(4):
                    nc.tensor.matmul(
                        po2[:, hs],
                        yt_sb[:, dc, tcn * P : (tcn + 1) * P],
                        wp_sb[:, dc, hs],
                        start=(dc == 0),
                        stop=(dc == 3),
                    )
                if use_act:  # ACT is idle at the tail; DVE is not
                    nc.scalar.activation(ob[:, hs], po2[:, hs], AFT.Copy)
                else:
                    nc.vector.tensor_copy(out=ob[:, hs], in_=po2[:, hs])
            nc.sync.dma_start(out_r[:, tcn, :], ob[:])

        xtbs = {}

        def get_xtb(jq):
            if jq not in xtbs:
                xtbs[jq] = work.tile(
                    [P, NCO, 512], bf16, tag="xtb", bufs=2, name=f"xtb{jq}"
                )
                nc.sync.dma_start(xtbs[jq][:], xt_d[jq])
            return xtbs[jq]

        def emit_mgroup(jq, m):
            xtb = xtbs[jq]
            blk = slice(jq * 512, (jq + 1) * 512)
            pq = psum.tile([P, 1024], f32, tag="qk2", bufs=3)
            for w_sb, half in ((wq_sb, 0), (wk_sb, 1)):
                for co in range(NCO):
                    nc.tensor.matmul(
                        pq[:, half * 512 : (half + 1) * 512],
                        w_sb[:, m, co, :],
                        xtb[:, co, :],
                        start=(co == 0),
                        stop=(co == NCO - 1),
                    )
            nc.vector.tensor_copy(out=qkt_sb[:, :, m, blk], in_=g2(pq[:]))

        pulled = {jq: 0 for jq in range(NJQ)}  # m-groups emitted early

        for jq in range(NJQ):
            blk = slice(jq * 512, (jq + 1) * 512)
            # ---- projections for this q/t block ----
            if jq == 0:
                xtb = xtbs[0] = work.tile(
                    [P, NCO, 512], bf16, tag="xtb", bufs=2, name="xtb0"
                )
                for co in range(NCO):  # pipeline with the first matmuls
                    nc.sync.dma_start(xtb[:, co, :], xt_d[0, :, co, :])
                nc.sync.dma_start(wk_sb[:, 0], wk_d[0])
                for m in range(1, 4):
                    nc.sync.dma_start(wq_sb[:, m], wq_d[m])
                    nc.sync.dma_start(wk_sb[:, m], wk_d[m])
                nc.sync.dma_start(wv_sb[:], wv_d[:])
                nc.sync.dma_start(tri_sb[:], tri_d[:])
                nc.sync.dma_start(bcm_sb[:], bcm_d[:])
            else:
                xtb = get_xtb(jq)
            if jq == 1:
                nc.sync.dma_start(wp_sb[:], wp_r)
            for m in range(pulled[jq], 4):  # qt/kt row chunks of D
                emit_mgroup(jq, m)
            # norm of the previous block: its DVE muls run while the PE
            # does the v projections, so the interleaved outproj below
            # never waits on them
            if jq > 0:
                emit_norm(jq - 1)
            for t4 in range(4):  # v chunks of 128 t-rows
                kc = jq * 4 + t4
                pv = psum.tile([P, 512], f32, tag=("pe" if t4 % 2 == 0 else "po"), bufs=1)
                for co in range(NCO):
                    nc.tensor.matmul(
                        pv[:],
                        xtb[:, co, t4 * P : (t4 + 1) * P],
                        wv_sb[:, co, :],
                        start=(co == 0),
                        stop=(co == NCO - 1),
                    )
                src = pv[:].rearrange("p (hp par c) -> p hp par c", par=2, c=64)
                nc.vector.tensor_copy(
                    out=v5[:, kc, :, 0, 0:64], in_=src[:, :, 0, :]
                )
                nc.vector.tensor_copy(
                    out=v5[:, kc, :, 1, 64:128], in_=src[:, :, 1, :]
                )

            # ---- attention for this q block ----
            for hp in range(4):
                # deferred outproj blocks + next block's first proj
                # m-groups fill the PE's exp-wait gaps (attention is
                # ACT-bound); each insertion kept near the ~2-chunk
                # exp-pipeline depth so the ACT stream never runs dry
                if hp < 2:
                    if jq > 0:
                        emit_outproj_tcn(jq - 1, 2 * hp)
                        emit_outproj_tcn(jq - 1, 2 * hp + 1)
                else:
                    if jq < NJQ - 1:
                        get_xtb(jq + 1)
                        emit_mgroup(jq + 1, hp - 2)
                        pulled[jq + 1] = hp - 1
                    elif hp == 3:
                        # last block: recip for hp0-2 queued on DVE now; the
                        # PE-side norm pieces land a few chunks into hp3's
                        # attention so the PE never waits on this chain
                        nc.vector.reciprocal_approx_fast(
                            out=scr_sb[0:7, jq, :], in_=sc_sb[0:7, jq, :]
                        )
                        nc.vector.tensor_copy(
                            out=scb_sb[0:7, jq, :], in_=scr_sb[0:7, jq, :]
                        )
                nk = (jq + 1) * 4
                psyE = psum.tile([P, 512], f32, tag="pe", bufs=1)
                psyO = psum.tile([P, 512], f32, tag="po", bufs=1)

                def emit_av(kc, att, off, stop):
                    ga = g2(att[:])
                    nc.tensor.matmul(
                        psyE[:, off:512], v_sb[:, kc, 2 * hp, :],
                        ga[:, 0, off:512], start=(kc == 0), stop=stop,
                    )
                    nc.tensor.matmul(
                        psyO[:, off:512], v_sb[:, kc, 2 * hp + 1, :],
                        ga[:, 1, off:512], start=(kc == 0), stop=stop,
                    )

                pend = []  # software-pipelined AV emission (lag 1)
                for kc in range(nk):
                    if kc == 5 and jq == NJQ - 1 and hp == 3:
                        for h2 in range(3):  # normalize hp0-2 mid-attention
                            emit_norm_hp(jq, h2)
                    d = kc - jq * 4
                    off = P * d if d >= 0 else 0
                    att = work.tile([P, 1024], bf16, tag="att", bufs=6)
                    ps = psum.tile([P, 1024], f32, tag="qk2", bufs=3)
                    for par, sl in ((0, slice(0, 64)), (1, slice(64, 128))):
                        nc.tensor.matmul(
                            ps[:, par * 512 + off : (par + 1) * 512],
                            qkt_sb[sl, 1, hp, kc * P : (kc + 1) * P],
                            qkt_sb[sl, 0, hp, jq * 512 + off : (jq + 1) * 512],
                            start=True,
                            stop=True,
                        )
                    nc.scalar.activation(
                        g2(att[:])[:, :, off:512], g2(ps[:])[:, :, off:512],
                        AFT.Exp, scale=0.125,
                    )
                    if d >= 0:
                        nc.vector.tensor_mul(
                            out=g2(att[:])[:, :, off : off + P],
                            in0=g2(att[:])[:, :, off : off + P],
                            in1=tri_sb[:],
                        )
                    pend.append((kc, att, off))
                    if len(pend) > 1:
                        emit_av(*pend.pop(0), stop=False)
                for i, pv in enumerate(pend):
                    emit_av(*pv, stop=(i == len(pend) - 1))

                # s rows first (they gate the deferred norm's recip chain):
                # copy to SBUF rows 0/64, then DMA-gather onto compact
                # partitions hp / 4+hp of sc (cross-partition)
                # s copies on ACT: it has a natural lull at the head-pair
                # boundary while DVE is congested with yt/ob copies
                nc.scalar.activation(s2_sb[64:65, hp, :], psyE[64:65, :], AFT.Copy)
                nc.scalar.activation(s2_sb[0:1, hp, :], psyO[0:1, :], AFT.Copy)
                nc.sync.dma_start(sc_sb[hp : hp + 1, jq, :], s2_sb[0:1, hp, :])
                nc.sync.dma_start(
                    sc_sb[4 + hp : 5 + hp, jq, :], s2_sb[64:65, hp, :]
                )
                nc.vector.tensor_copy(
                    out=yt_sb[0:64, hp, blk], in_=psyE[0:64, :]
                )
                nc.vector.tensor_copy(
                    out=yt_sb[64:128, hp, blk], in_=psyO[64:128, :]
                )

        # tail: tcn 12/13 accumulate dc0-2 (hp0-2 already normalized)
        # while the DVE recip chain for hp3 completes; only the dc3
        # matmuls wait on it
        jql = NJQ - 1
        po_ab = []
        for t4 in range(2):
            tcn = jql * 4 + t4
            po2 = psum.tile([P, 1024], f32, tag="qk2", bufs=3)
            po_ab.append(po2)
            for half in range(2):
                for dc in range(3):
                    nc.tensor.matmul(
                        po2[:, half * 512 : (half + 1) * 512],
                        yt_sb[:, dc, tcn * P : (tcn + 1) * P],
                        wp_sb[:, dc, half * 512 : (half + 1) * 512],
                        start=(dc == 0), stop=False,
                    )
        emit_recip(jql)
        emit_norm_hp(jql, 3)
        for t4 in range(2):
            tcn = jql * 4 + t4
            po2 = po_ab[t4]
            for half in range(2):
                nc.tensor.matmul(
                    po2[:, half * 512 : (half + 1) * 512],
                    yt_sb[:, 3, tcn * P : (tcn + 1) * P],
                    wp_sb[:, 3, half * 512 : (half + 1) * 512],
                    start=False, stop=True,
                )
            ob = work.tile([P, C], bf16, tag="ob", bufs=2)
            nc.scalar.activation(ob[:], po2[:], AFT.Copy)
            nc.sync.dma_start(out_r[:, tcn, :], ob[:])
        emit_outproj_tcn(jql, 2, use_act=True)
        emit_outproj_tcn(jql, 3, use_act=True)

    nc.finalize()
    return nc


_NC = None


def _get_nc():
    global _NC
    if _NC is None:
        _NC = build_nc()
    return _NC


def make_in_maps(x, Wk, Wq, Wv, Wp):
    x = np.asarray(x, dtype=np.float32)
    Wk = np.asarray(Wk, dtype=np.float32)
    Wq = np.asarray(Wq, dtype=np.float32)
    Wv = np.asarray(Wv, dtype=np.float32)
    Wp = np.asarray(Wp, dtype=np.float32)
    tri1 = np.triu(np.ones((P, P), np.float32))
    tri = np.stack([tri1, tri1], axis=1).astype(ml_dtypes.bfloat16)  # [P,2,P]
    bcm = np.zeros((8, 4, P), np.float32)
    for hp in range(4):
        bcm[4 + hp, hp, 0:64] = 1.0    # even head recip -> yt rows 0:64
        bcm[hp, hp, 64:128] = 1.0      # odd head recip -> yt rows 64:128
    bcm = bcm.astype(ml_dtypes.bfloat16)

    def shuf_x(xb):  # [C,T] -> [jq, p, co, t-in-block]
        return np.ascontiguousarray(
            xb.T.reshape(NCO, P, NJQ, 512).transpose(2, 1, 0, 3)
        ).astype(ml_dtypes.bfloat16)

    def shuf_w(w):  # [C,D] -> [p, co, d]
        return np.ascontiguousarray(
            w.reshape(NCO, P, D).transpose(1, 0, 2)
        ).astype(ml_dtypes.bfloat16)

    def shuf_qk(w):  # [C,D] -> [m, p, co, dd]
        return np.ascontiguousarray(
            w.reshape(NCO, P, 4, P).transpose(2, 1, 0, 3)
        ).astype(ml_dtypes.bfloat16)

    xt_b = [shuf_x(x[b]) for b in range(B)]
    in_maps = []
    for c in range(8):
        b, g = c // 2, c % 2
        sl = slice(g * D, (g + 1) * D)
        in_maps.append({
            "xt": xt_b[b],
            "wq": shuf_qk(Wq[:, sl]),
            "wk": shuf_qk(Wk[:, sl]),
            "wv": shuf_w(Wv[:, sl]),
            "wp": np.ascontiguousarray(Wp[sl, :]).astype(ml_dtypes.bfloat16),
            "tri": tri,
            "bcm": bcm,
        })
    return in_maps


def _run(x, Wk, Wq, Wv, Wp, trace=False):
    nc = _get_nc()
    in_maps = make_in_maps(x, Wk, Wq, Wv, Wp)
    res = run_bass_kernel_spmd(nc, in_maps, core_ids=list(range(8)), trace=trace)
    parts = [res.results[c]["out"] for c in range(8)]
    out = np.stack(
        [parts[2 * b] + parts[2 * b + 1] for b in range(B)], axis=0
    ).astype(np.float32)
    return out, res


def kernel(x, Wk, Wq, Wv, Wp):
    out, _ = _run(x, Wk, Wq, Wv, Wp, trace=False)
    return out


# revision 50
# speedup vs baseline: 1.0099x; 1.0099x over previous
"""Causal self-attention (B=4, T=2048, C=1024, NH=16) on 8 TRN2 NeuronCores.

Sharding: core c -> batch b = c//2, head-group g = c%2 (8 heads, D=512).
Each core computes q/k/v projections for its head group on its batch,
fused causal attention (attT layout: k on partitions), and a partial
output projection through its row-slice of Wp. Host sums the two
partials per batch.

All operands bf16 (host pre-casts x/W): halves input DMA and enables
the compiler's automatic Fast Weight Load (FWL needs non-fp32 128-col
weights), so LDWEIGHTS overlaps the matmul stream.

Single fused loop over q-blocks jq (512 q each):
  proj(jq): qt/kt m-chunks into one [128,1024] psum (q|k halves, one
  DVE copy), v chunks scattered into per-k-chunk AV lhsT slots whose
  per-head ones column makes softmax denominators a free psum row.
  norm+outproj(jq-1): deferred one block so the PE never waits on the
  ACT recip chain (s rows live on partitions {hp} / {64+hp}; Ln+Exp on
  [4,512] tiles; PE bcm matmul broadcasts 1/s; one TT mul per (hp)).
  attn(jq): per head pair, QK chunk pair into [128,1024] psum, ONE exp
  per chunk ([2,512-off] strided AP), tri-mask on diagonal chunks,
  software-pipelined AV that streams only the valid [off:512] columns.

kernel(**inputs) takes the FULL unsharded inputs and returns the FULL
output. Self-contained: hardcodes all shapes, reads nothing from disk.
"""

import sys

sys.path.insert(0, "/opt/trn_rl_repo")

import numpy as np
import ml_dtypes
from contextlib import ExitStack

import concourse.bass as bass  # noqa: F401
import concourse.mybir as mybir
import concourse.tile as tile
from concourse import bacc
from concourse.bass_utils import run_bass_kernel_spmd

P = 128
B, T, C = 4, 2048, 1024
NH, HS = 16, 64
D = 512          # per-core head dim (8 heads)
H = 8            # local heads
NCO = C // P     # 8 contraction chunks
NKC = T // P     # 16 k chunks
NJQ = T // 512   # 4 q blocks
f32 = mybir.dt.float32
bf16 = mybir.dt.bfloat16
AFT = mybir.ActivationFunctionType


def g2(ap):
    """View a [P, 1024] AP as [P, 2, 512] (even|odd halves)."""
    return ap.rearrange("p (g c) -> p g c", g=2)


def build_nc():
    nc = bacc.Bacc("TRN2", target_bir_lowering=False, debug=False, num_devices=8)

    # host pre-shuffled layouts: per-partition bytes contiguous (4-8KB
    # DMA descriptors instead of 1KB rows)
    xt_d = nc.dram_tensor("xt", [NJQ, P, NCO, 512], bf16, kind="ExternalInput")
    wq_d = nc.dram_tensor("wq", [4, P, NCO, P], bf16, kind="ExternalInput")
    wk_d = nc.dram_tensor("wk", [4, P, NCO, P], bf16, kind="ExternalInput")
    wv_d = nc.dram_tensor("wv", [P, NCO, D], bf16, kind="ExternalInput")
    wp_d = nc.dram_tensor("wp", [D, C], bf16, kind="ExternalInput")
    tri_d = nc.dram_tensor("tri", [P, 2, P], bf16, kind="ExternalInput")
    bcm_d = nc.dram_tensor("bcm", [8, 4, P], bf16, kind="ExternalInput")
    out_d = nc.dram_tensor("out", [T, C], bf16, kind="ExternalOutput")

    wp_r = wp_d[:].rearrange("(dc p) c -> p dc c", p=P)
    out_r = out_d[:].rearrange("(tc p) c -> p tc c", p=P)

    with tile.TileContext(nc) as tc, ExitStack() as ctx, nc.allow_low_precision(
        reason="bf16 attention kernel"
    ):
        perm = ctx.enter_context(tc.tile_pool(name="perm", bufs=1))
        work = ctx.enter_context(tc.tile_pool(name="work", bufs=1))
        psum = ctx.enter_context(tc.tile_pool(name="psum", bufs=2, space="PSUM"))

        wq_sb = perm.tile([P, 4, NCO, P], bf16)  # [p, m, co, dd]
        qkt_sb = perm.tile([P, 2, 4, T], bf16)   # [dh-pair, q|k, hp, t]
        wk_sb = perm.tile([P, 4, NCO, P], bf16)
        wv_sb = perm.tile([P, NCO, D], bf16)
        wp_sb = perm.tile([P, 4, C], bf16)
        v_sb = perm.tile([P, NKC, H, P], bf16)   # per-chunk AV lhsT slots
        yt_sb = perm.tile([P, 4, T], bf16)
        s2_sb = perm.tile([P, 4, 512], f32)      # raw s rows 0 (odd) / 64 (even)
        sc_sb = perm.tile([8, NJQ, 512], f32)    # gathered s, 8 partitions
        scr_sb = perm.tile([8, NJQ, 512], f32)   # 1/s
        scb_sb = perm.tile([8, NJQ, 512], bf16)  # 1/s as bf16 matmul operand
        tri_sb = perm.tile([P, 2, P], bf16)
        bcm_sb = perm.tile([8, 4, P], bf16)

        # earliest-needed first: wq m0 + first x co-chunk gate the first
        # matmul; the rest is emitted inside the loop, most-urgent first
        nc.sync.dma_start(wq_sb[:, 0], wq_d[0])

        dum_sb = perm.tile([P, 512], bf16)
        nc.gpsimd.memset(dum_sb[:], 0.0)
        # warm the PE p-state during the DMA-init window: ~40 back-to-back
        # dummy matmuls ramp the clock to 2.4GHz before real data lands
        for i in range(12):
            pd = psum.tile([P, 512], f32, tag=("pe" if i % 2 == 0 else "po"),
                           bufs=1, name=f"dum{i}")
            nc.tensor.matmul(
                pd[:], dum_sb[:, 0:128], dum_sb[:], start=True, stop=True
            )

        nc.gpsimd.memset(v_sb[:], 0.0)
        nc.gpsimd.memset(sc_sb[:], 1.0)
        nc.gpsimd.memset(scb_sb[:], 1.0)
        v5 = v_sb[:].rearrange("p k (hp par) c -> p k hp par c", par=2)
        # ones cols: even head -> col 64 (s at psum row 64); odd -> col 0
        nc.gpsimd.memset(v5[:, :, :, 0, 64:65], 1.0)
        nc.gpsimd.memset(v5[:, :, :, 1, 0:1], 1.0)

        def emit_recip(jq):
            nc.vector.reciprocal_approx_fast(
                out=scr_sb[0:8, jq, :], in_=sc_sb[0:8, jq, :]
            )
            nc.vector.tensor_copy(out=scb_sb[0:8, jq, :], in_=scr_sb[0:8, jq, :])

        def emit_norm(jq):
            blk = slice(jq * 512, (jq + 1) * 512)
            emit_recip(jq)
            for pair in range(2):
                rb2 = psum.tile([P, 1024], f32, tag="qk2", bufs=3)
                for half in range(2):
                    hp = 2 * pair + half
                    nc.tensor.matmul(
                        rb2[:, half * 512 : (half + 1) * 512],
                        bcm_sb[0:8, hp, :], scb_sb[0:8, jq, :],
                        start=True, stop=True,
                    )
                for half in range(2):
                    hp = 2 * pair + half
                    nc.vector.tensor_mul(
                        out=yt_sb[:, hp, blk], in0=yt_sb[:, hp, blk],
                        in1=g2(rb2[:])[:, half, :],
                    )

        def emit_norm_hp(jq, hp):
            blk = slice(jq * 512, (jq + 1) * 512)
            rbh = psum.tile([P, 1024], f32, tag="qk2", bufs=3)
            nc.tensor.matmul(
                rbh[:, 0:512], bcm_sb[0:8, hp, :], scb_sb[0:8, jq, :],
                start=True, stop=True,
            )
            nc.vector.tensor_mul(
                out=yt_sb[:, hp, blk], in0=yt_sb[:, hp, blk],
                in1=rbh[:, 0:512],
            )

        def emit_outproj_tcn(jq, t4, use_act=False):
            tcn = jq * 4 + t4
            po2 = psum.tile([P, 1024], f32, tag="qk2", bufs=3)
            ob = work.tile([P, C], bf16, tag="ob", bufs=2)
            # per-half psum->sbuf copies, emitted right after each half's
            # last matmul: frees the psum buffer sooner (the next QK pair
            # WARs this tile) and overlaps copy with the other half
            for half in range(2):
                hs = slice(half * 512, (half + 1) * 512)
                for dc in range(4):
                    nc.tensor.matmul(
                        po2[:, hs],
                        yt_sb[:, dc, tcn * P : (tcn + 1) * P],
                        wp_sb[:, dc, hs],
                        start=(dc == 0),
                        stop=(dc == 3),
                    )
                if use_act:  # ACT is idle at the tail; DVE is not
                    nc.scalar.activation(ob[:, hs], po2[:, hs], AFT.Copy)
                else:
                    nc.vector.tensor_copy(out=ob[:, hs], in_=po2[:, hs])
            nc.sync.dma_start(out_r[:, tcn, :], ob[:])

        xtbs = {}

        def get_xtb(jq):
            if jq not in xtbs:
                xtbs[jq] = work.tile(
                    [P, NCO, 512], bf16, tag="xtb", bufs=2, name=f"xtb{jq}"
                )
                nc.sync.dma_start(xtbs[jq][:], xt_d[jq])
            return xtbs[jq]

        def emit_mgroup(jq, m):
            xtb = xtbs[jq]
            blk = slice(jq * 512, (jq + 1) * 512)
            pq = psum.tile([P, 1024], f32, tag="qk2", bufs=3)
            for w_sb, half in ((wq_sb, 0), (wk_sb, 1)):
                for co in range(NCO):
                    nc.tensor.matmul(
                        pq[:, half * 512 : (half + 1) * 512],
                        w_sb[:, m, co, :],
                        xtb[:, co, :],
                        start=(co == 0),
                        stop=(co == NCO - 1),
                    )
            nc.vector.tensor_copy(out=qkt_sb[:, :, m, blk], in_=g2(pq[:]))

        pulled = {jq: 0 for jq in range(NJQ)}  # m-groups emitted early

        for jq in range(NJQ):
            blk = slice(jq * 512, (jq + 1) * 512)
            # ---- projections for this q/t block ----
            if jq == 0:
                xtb = xtbs[0] = work.tile(
                    [P, NCO, 512], bf16, tag="xtb", bufs=2, name="xtb0"
                )
                for co in range(NCO):  # pipeline with the first matmuls
                    nc.sync.dma_start(xtb[:, co, :], xt_d[0, :, co, :])
                nc.sync.dma_start(wk_sb[:, 0], wk_d[0])
                for m in range(1, 4):
                    nc.sync.dma_start(wq_sb[:, m], wq_d[m])
                    nc.sync.dma_start(wk_sb[:, m], wk_d[m])
                nc.sync.dma_start(wv_sb[:], wv_d[:])
                nc.sync.dma_start(tri_sb[:], tri_d[:])
                nc.sync.dma_start(bcm_sb[:], bcm_d[:])
            else:
                xtb = get_xtb(jq)
            if jq == 1:
                nc.sync.dma_start(wp_sb[:], wp_r)
            for m in range(pulled[jq], 4):  # qt/kt row chunks of D
                emit_mgroup(jq, m)
            # norm of the previous block: its DVE muls run while the PE
            # does the v projections, so the interleaved outproj below
            # never waits on them
            if jq > 0:
                emit_norm(jq - 1)
            for t4 in range(4):  # v chunks of 128 t-rows
                kc = jq * 4 + t4
                pv = psum.tile([P, 512], f32, tag=("pe" if t4 % 2 == 0 else "po"), bufs=1)
                for co in range(NCO):
                    nc.tensor.matmul(
                        pv[:],
                        xtb[:, co, t4 * P : (t4 + 1) * P],
                        wv_sb[:, co, :],
                        start=(co == 0),
                        stop=(co == NCO - 1),
                    )
                src = pv[:].rearrange("p (hp par c) -> p hp par c", par=2, c=64)
                nc.vector.tensor_copy(
                    out=v5[:, kc, :, 0, 0:64], in_=src[:, :, 0, :]
                )
                nc.vector.tensor_copy(
                    out=v5[:, kc, :, 1, 64:128], in_=src[:, :, 1, :]
                )

            # ---- attention for this q block ----
            for hp in range(4):
                # deferred outproj blocks + next block's first proj
                # m-groups fill the PE's exp-wait gaps (attention is
                # ACT-bound); each insertion kept near the ~2-chunk
                # exp-pipeline depth so the ACT stream never runs dry
                if hp >= 2:
                    if jq < NJQ - 1:
                        get_xtb(jq + 1)
                        emit_mgroup(jq + 1, hp - 2)
                        pulled[jq + 1] = hp - 1
                    elif hp == 3:
                        # last block: recip for hp0-2 queued on DVE now; the
                        # PE-side norm pieces land a few chunks into hp3's
                        # attention so the PE never waits on this chain
                        nc.vector.reciprocal_approx_fast(
                            out=scr_sb[0:7, jq, :], in_=sc_sb[0:7, jq, :]
                        )
                        nc.vector.tensor_copy(
                            out=scb_sb[0:7, jq, :], in_=scr_sb[0:7, jq, :]
                        )
                nk = (jq + 1) * 4
                psyE = psum.tile([P, 512], f32, tag="pe", bufs=1)
                psyO = psum.tile([P, 512], f32, tag="po", bufs=1)

                def emit_av(kc, att, off, stop):
                    ga = g2(att[:])
                    nc.tensor.matmul(
                        psyE[:, off:512], v_sb[:, kc, 2 * hp, :],
                        ga[:, 0, off:512], start=(kc == 0), stop=stop,
                    )
                    nc.tensor.matmul(
                        psyO[:, off:512], v_sb[:, kc, 2 * hp + 1, :],
                        ga[:, 1, off:512], start=(kc == 0), stop=stop,
                    )

                pend = []  # software-pipelined AV emission (lag 1)
                for kc in range(nk):
                    if kc == 5 and jq == NJQ - 1 and hp == 3:
                        for h2 in range(3):  # normalize hp0-2 mid-attention
                            emit_norm_hp(jq, h2)
                    d = kc - jq * 4
                    off = P * d if d >= 0 else 0
                    att = work.tile([P, 1024], bf16, tag="att", bufs=6)
                    ps = psum.tile([P, 1024], f32, tag="qk2", bufs=3)
                    for par, sl in ((0, slice(0, 64)), (1, slice(64, 128))):
                        nc.tensor.matmul(
                            ps[:, par * 512 + off : (par + 1) * 512],
                            qkt_sb[sl, 1, hp, kc * P : (kc + 1) * P],
                            qkt_sb[sl, 0, hp, jq * 512 + off : (jq + 1) * 512],
                            start=True,
                            stop=True,
                        )
                    nc.scalar.activation(
                        g2(att[:])[:, :, off:512], g2(ps[:])[:, :, off:512],
                        AFT.Exp, scale=0.125,
                    )
                    if d >= 0:
                        nc.vector.tensor_mul(
                            out=g2(att[:])[:, :, off : off + P],
                            in0=g2(att[:])[:, :, off : off + P],
                            in1=tri_sb[:],
                        )
                    pend.append((kc, att, off))
                    if len(pend) > 1:
                        emit_av(*pend.pop(0), stop=False)
                    if kc == 1 and hp < 2 and jq > 0:
                        # two deferred outproj blocks after the first AV:
                        # their psum WAR and DVE ob copies clear well before
                        # the next head-pair boundary
                        emit_outproj_tcn(jq - 1, 2 * hp)
                        emit_outproj_tcn(jq - 1, 2 * hp + 1)
                for i, pv in enumerate(pend):
                    emit_av(*pv, stop=(i == len(pend) - 1))

                # s rows first (they gate the deferred norm's recip chain):
                # copy to SBUF rows 0/64, then DMA-gather onto compact
                # partitions hp / 4+hp of sc (cross-partition)
                nc.vector.tensor_copy(
                    out=s2_sb[64:65, hp, :], in_=psyE[64:65, :]
                )
                nc.vector.tensor_copy(
                    out=s2_sb[0:1, hp, :], in_=psyO[0:1, :]
                )
                nc.sync.dma_start(sc_sb[hp : hp + 1, jq, :], s2_sb[0:1, hp, :])
                nc.sync.dma_start(
                    sc_sb[4 + hp : 5 + hp, jq, :], s2_sb[64:65, hp, :]
                )
                nc.vector.tensor_copy(
                    out=yt_sb[0:64, hp, blk], in_=psyE[0:64, :]
                )
                nc.vector.tensor_copy(
                    out=yt_sb[64:128, hp, blk], in_=psyO[64:128, :]
                )

        # tail: tcn 12/13 accumulate dc0-2 (hp0-2 already normalized)
        # while the DVE recip chain for hp3 completes; only the dc3
        # matmuls wait on it
        jql = NJQ - 1
        po_ab = []
        for t4 in range(2):
            tcn = jql * 4 + t4
            po2 = psum.tile([P, 1024], f32, tag="qk2", bufs=3)
            po_ab.append(po2)
            for half in range(2):
                for dc in range(3):
                    nc.tensor.matmul(
                        po2[:, half * 512 : (half + 1) * 512],
                        yt_sb[:, dc, tcn * P : (tcn + 1) * P],
                        wp_sb[:, dc, half * 512 : (half + 1) * 512],
                        start=(dc == 0), stop=False,
                    )
        emit_recip(jql)
        emit_norm_hp(jql, 3)
        for t4 in range(2):
            tcn = jql * 4 + t4
            po2 = po_ab[t4]
            for half in range(2):
                nc.tensor.matmul(
                    po2[:, half * 512 : (half + 1) * 512],
                    yt_sb[:, 3, tcn * P : (tcn + 1) * P],
                    wp_sb[:, 3, half * 512 : (half + 1) * 512],
                    start=False, stop=True,
                )
            ob = work.tile([P, C], bf16, tag="ob", bufs=2)
            nc.scalar.activation(ob[:], po2[:], AFT.Copy)
            nc.sync.dma_start(out_r[:, tcn, :], ob[:])
        emit_outproj_tcn(jql, 2, use_act=True)
        emit_outproj_tcn(jql, 3, use_act=True)

    nc.finalize()
    return nc


_NC = None


def _get_nc():
    global _NC
    if _NC is None:
        _NC = build_nc()
    return _NC


def make_in_maps(x, Wk, Wq, Wv, Wp):
    x = np.asarray(x, dtype=np.float32)
    Wk = np.asarray(Wk, dtype=np.float32)
    Wq = np.asarray(Wq, dtype=np.float32)
    Wv = np.asarray(Wv, dtype=np.float32)
    Wp = np.asarray(Wp, dtype=np.float32)
    tri1 = np.triu(np.ones((P, P), np.float32))
    tri = np.stack([tri1, tri1], axis=1).astype(ml_dtypes.bfloat16)  # [P,2,P]
    bcm = np.zeros((8, 4, P), np.float32)
    for hp in range(4):
        bcm[4 + hp, hp, 0:64] = 1.0    # even head recip -> yt rows 0:64
        bcm[hp, hp, 64:128] = 1.0      # odd head recip -> yt rows 64:128
    bcm = bcm.astype(ml_dtypes.bfloat16)

    def shuf_x(xb):  # [C,T] -> [jq, p, co, t-in-block]
        return np.ascontiguousarray(
            xb.T.reshape(NCO, P, NJQ, 512).transpose(2, 1, 0, 3)
        ).astype(ml_dtypes.bfloat16)

    def shuf_w(w):  # [C,D] -> [p, co, d]
        return np.ascontiguousarray(
            w.reshape(NCO, P, D).transpose(1, 0, 2)
        ).astype(ml_dtypes.bfloat16)

    def shuf_qk(w):  # [C,D] -> [m, p, co, dd]
        return np.ascontiguousarray(
            w.reshape(NCO, P, 4, P).transpose(2, 1, 0, 3)
        ).astype(ml_dtypes.bfloat16)

    xt_b = [shuf_x(x[b]) for b in range(B)]
    in_maps = []
    for c in range(8):
        b, g = c // 2, c % 2
        sl = slice(g * D, (g + 1) * D)
        in_maps.append({
            "xt": xt_b[b],
            "wq": shuf_qk(Wq[:, sl]),
            "wk": shuf_qk(Wk[:, sl]),
            "wv": shuf_w(Wv[:, sl]),
            "wp": np.ascontiguousarray(Wp[sl, :]).astype(ml_dtypes.bfloat16),
            "tri": tri,
            "bcm": bcm,
        })
    return in_maps


def _run(x, Wk, Wq, Wv, Wp, trace=False):
    nc = _get_nc()
    in_maps = make_in_maps(x, Wk, Wq, Wv, Wp)
    res = run_bass_kernel_spmd(nc, in_maps, core_ids=list(range(8)), trace=trace)
    parts = [res.results[c]["out"] for c in range(8)]
    out = np.stack(
        [parts[2 * b] + parts[2 * b + 1] for b in range(B)], axis=0
    ).astype(np.float32)
    return out, res


def kernel(x, Wk, Wq, Wv, Wp):
    out, _ = _run(x, Wk, Wq, Wv, Wp, trace=False)
    return out


# revision 51
# speedup vs baseline: 1.0153x; 1.0053x over previous
"""Causal self-attention (B=4, T=2048, C=1024, NH=16) on 8 TRN2 NeuronCores.

Sharding: core c -> batch b = c//2, head-group g = c%2 (8 heads, D=512).
Each core computes q/k/v projections for its head group on its batch,
fused causal attention (attT layout: k on partitions), and a partial
output projection through its row-slice of Wp. Host sums the two
partials per batch.

All operands bf16 (host pre-casts x/W): halves input DMA and enables
the compiler's automatic Fast Weight Load (FWL needs non-fp32 128-col
weights), so LDWEIGHTS overlaps the matmul stream.

Single fused loop over q-blocks jq (512 q each):
  proj(jq): qt/kt m-chunks into one [128,1024] psum (q|k halves, one
  DVE copy), v chunks scattered into per-k-chunk AV lhsT slots whose
  per-head ones column makes softmax denominators a free psum row.
  norm+outproj(jq-1): deferred one block so the PE never waits on the
  ACT recip chain (s rows live on partitions {hp} / {64+hp}; Ln+Exp on
  [4,512] tiles; PE bcm matmul broadcasts 1/s; one TT mul per (hp)).
  attn(jq): per head pair, QK chunk pair into [128,1024] psum, ONE exp
  per chunk ([2,512-off] strided AP), tri-mask on diagonal chunks,
  software-pipelined AV that streams only the valid [off:512] columns.

kernel(**inputs) takes the FULL unsharded inputs and returns the FULL
output. Self-contained: hardcodes all shapes, reads nothing from disk.
"""

import sys

sys.path.insert(0, "/opt/trn_rl_repo")

import numpy as np
import ml_dtypes
from contextlib import ExitStack

import concourse.bass as bass  # noqa: F401
import concourse.mybir as mybir
import concourse.tile as tile
from concourse import bacc
from concourse.bass_utils import run_bass_kernel_spmd

P = 128
B, T, C = 4, 2048, 1024
NH, HS = 16, 64
D = 512          # per-core head dim (8 heads)
H = 8            # local heads
NCO = C // P     # 8 contraction chunks
NKC = T // P     # 16 k chunks
NJQ = T // 512   # 4 q blocks
f32 = mybir.dt.float32
bf16 = mybir.dt.bfloat16
AFT = mybir.ActivationFunctionType


def g2(ap):
    """View a [P, 1024] AP as [P, 2, 512] (even|odd halves)."""
    return ap.rearrange("p (g c) -> p g c", g=2)


def build_nc():
    nc = bacc.Bacc("TRN2", target_bir_lowering=False, debug=False, num_devices=8)

    # host pre-shuffled layouts: per-partition bytes contiguous (4-8KB
    # DMA descriptors instead of 1KB rows)
    xt_d = nc.dram_tensor("xt", [NJQ, P, NCO, 512], bf16, kind="ExternalInput")
    wq_d = nc.dram_tensor("wq", [4, P, NCO, P], bf16, kind="ExternalInput")
    wk_d = nc.dram_tensor("wk", [4, P, NCO, P], bf16, kind="ExternalInput")
    wv_d = nc.dram_tensor("wv", [P, NCO, D], bf16, kind="ExternalInput")
    wp_d = nc.dram_tensor("wp", [D, C], bf16, kind="ExternalInput")
    tri_d = nc.dram_tensor("tri", [P, 2, P], bf16, kind="ExternalInput")
    bcm_d = nc.dram_tensor("bcm", [8, 4, P], bf16, kind="ExternalInput")
    out_d = nc.dram_tensor("out", [T, C], bf16, kind="ExternalOutput")

    wp_r = wp_d[:].rearrange("(dc p) c -> p dc c", p=P)
    out_r = out_d[:].rearrange("(tc p) c -> p tc c", p=P)

    with tile.TileContext(nc) as tc, ExitStack() as ctx, nc.allow_low_precision(
        reason="bf16 attention kernel"
    ):
        perm = ctx.enter_context(tc.tile_pool(name="perm", bufs=1))
        work = ctx.enter_context(tc.tile_pool(name="work", bufs=1))
        psum = ctx.enter_context(tc.tile_pool(name="psum", bufs=2, space="PSUM"))

        wq_sb = perm.tile([P, 4, NCO, P], bf16)  # [p, m, co, dd]
        qkt_sb = perm.tile([P, 2, 4, T], bf16)   # [dh-pair, q|k, hp, t]
        wk_sb = perm.tile([P, 4, NCO, P], bf16)
        wv_sb = perm.tile([P, NCO, D], bf16)
        wp_sb = perm.tile([P, 4, C], bf16)
        v_sb = perm.tile([P, NKC, H, P], bf16)   # per-chunk AV lhsT slots
        yt_sb = perm.tile([P, 4, T], bf16)
        s2_sb = perm.tile([P, 4, 512], f32)      # raw s rows 0 (odd) / 64 (even)
        sc_sb = perm.tile([8, NJQ, 512], f32)    # gathered s, 8 partitions
        scr_sb = perm.tile([8, NJQ, 512], f32)   # 1/s
        scb_sb = perm.tile([8, NJQ, 512], bf16)  # 1/s as bf16 matmul operand
        tri_sb = perm.tile([P, 2, P], bf16)
        bcm_sb = perm.tile([8, 4, P], bf16)

        # earliest-needed first: wq m0 + first x co-chunk gate the first
        # matmul; the rest is emitted inside the loop, most-urgent first
        nc.sync.dma_start(wq_sb[:, 0], wq_d[0])

        dum_sb = perm.tile([P, 512], bf16)
        nc.gpsimd.memset(dum_sb[:], 0.0)
        # warm the PE p-state during the DMA-init window: ~40 back-to-back
        # dummy matmuls ramp the clock to 2.4GHz before real data lands
        for i in range(9):
            pd = psum.tile([P, 512], f32, tag=("pe" if i % 2 == 0 else "po"),
                           bufs=1, name=f"dum{i}")
            nc.tensor.matmul(
                pd[:], dum_sb[:, 0:128], dum_sb[:], start=True, stop=True
            )

        nc.gpsimd.memset(v_sb[:], 0.0)
        nc.gpsimd.memset(sc_sb[:], 1.0)
        nc.gpsimd.memset(scb_sb[:], 1.0)
        v5 = v_sb[:].rearrange("p k (hp par) c -> p k hp par c", par=2)
        # ones cols: even head -> col 64 (s at psum row 64); odd -> col 0
        nc.gpsimd.memset(v5[:, :, :, 0, 64:65], 1.0)
        nc.gpsimd.memset(v5[:, :, :, 1, 0:1], 1.0)

        def emit_recip(jq):
            nc.vector.reciprocal_approx_fast(
                out=scr_sb[0:8, jq, :], in_=sc_sb[0:8, jq, :]
            )
            nc.vector.tensor_copy(out=scb_sb[0:8, jq, :], in_=scr_sb[0:8, jq, :])

        def emit_norm(jq):
            blk = slice(jq * 512, (jq + 1) * 512)
            emit_recip(jq)
            for pair in range(2):
                rb2 = psum.tile([P, 1024], f32, tag="qk2", bufs=3)
                for half in range(2):
                    hp = 2 * pair + half
                    nc.tensor.matmul(
                        rb2[:, half * 512 : (half + 1) * 512],
                        bcm_sb[0:8, hp, :], scb_sb[0:8, jq, :],
                        start=True, stop=True,
                    )
                for half in range(2):
                    hp = 2 * pair + half
                    nc.vector.tensor_mul(
                        out=yt_sb[:, hp, blk], in0=yt_sb[:, hp, blk],
                        in1=g2(rb2[:])[:, half, :],
                    )

        def emit_norm_hp(jq, hp):
            blk = slice(jq * 512, (jq + 1) * 512)
            rbh = psum.tile([P, 1024], f32, tag="qk2", bufs=3)
            nc.tensor.matmul(
                rbh[:, 0:512], bcm_sb[0:8, hp, :], scb_sb[0:8, jq, :],
                start=True, stop=True,
            )
            nc.vector.tensor_mul(
                out=yt_sb[:, hp, blk], in0=yt_sb[:, hp, blk],
                in1=rbh[:, 0:512],
            )

        def emit_outproj_tcn(jq, t4, use_act=False):
            tcn = jq * 4 + t4
            po2 = psum.tile([P, 1024], f32, tag="qk2", bufs=3)
            ob = work.tile([P, C], bf16, tag="ob", bufs=2)
            # per-half psum->sbuf copies, emitted right after each half's
            # last matmul: frees the psum buffer sooner (the next QK pair
            # WARs this tile) and overlaps copy with the other half
            for half in range(2):
                hs = slice(half * 512, (half + 1) * 512)
                for dc in range(4):
                    nc.tensor.matmul(
                        po2[:, hs],
                        yt_sb[:, dc, tcn * P : (tcn + 1) * P],
                        wp_sb[:, dc, hs],
                        start=(dc == 0),
                        stop=(dc == 3),
                    )
                if use_act:  # ACT is idle at the tail; DVE is not
                    nc.scalar.activation(ob[:, hs], po2[:, hs], AFT.Copy)
                else:
                    nc.vector.tensor_copy(out=ob[:, hs], in_=po2[:, hs])
            nc.sync.dma_start(out_r[:, tcn, :], ob[:])

        xtbs = {}

        def get_xtb(jq):
            if jq not in xtbs:
                xtbs[jq] = work.tile(
                    [P, NCO, 512], bf16, tag="xtb", bufs=2, name=f"xtb{jq}"
                )
                nc.sync.dma_start(xtbs[jq][:], xt_d[jq])
            return xtbs[jq]

        def emit_mgroup(jq, m):
            xtb = xtbs[jq]
            blk = slice(jq * 512, (jq + 1) * 512)
            pq = psum.tile([P, 1024], f32, tag="qk2", bufs=3)
            for w_sb, half in ((wq_sb, 0), (wk_sb, 1)):
                for co in range(NCO):
                    nc.tensor.matmul(
                        pq[:, half * 512 : (half + 1) * 512],
                        w_sb[:, m, co, :],
                        xtb[:, co, :],
                        start=(co == 0),
                        stop=(co == NCO - 1),
                    )
            nc.vector.tensor_copy(out=qkt_sb[:, :, m, blk], in_=g2(pq[:]))

        pulled = {jq: 0 for jq in range(NJQ)}  # m-groups emitted early

        for jq in range(NJQ):
            blk = slice(jq * 512, (jq + 1) * 512)
            # ---- projections for this q/t block ----
            if jq == 0:
                xtb = xtbs[0] = work.tile(
                    [P, NCO, 512], bf16, tag="xtb", bufs=2, name="xtb0"
                )
                for co in range(NCO):  # pipeline with the first matmuls
                    nc.sync.dma_start(xtb[:, co, :], xt_d[0, :, co, :])
                nc.sync.dma_start(wk_sb[:, 0], wk_d[0])
                for m in range(1, 4):
                    nc.sync.dma_start(wq_sb[:, m], wq_d[m])
                    nc.sync.dma_start(wk_sb[:, m], wk_d[m])
                nc.sync.dma_start(wv_sb[:], wv_d[:])
                nc.sync.dma_start(tri_sb[:], tri_d[:])
                nc.sync.dma_start(bcm_sb[:], bcm_d[:])
            else:
                xtb = get_xtb(jq)
            if jq == 1:
                nc.sync.dma_start(wp_sb[:], wp_r)
            for m in range(pulled[jq], 4):  # qt/kt row chunks of D
                emit_mgroup(jq, m)
            # norm of the previous block: its DVE muls run while the PE
            # does the v projections, so the interleaved outproj below
            # never waits on them
            if jq > 0:
                emit_norm(jq - 1)
            for t4 in range(4):  # v chunks of 128 t-rows
                kc = jq * 4 + t4
                pv = psum.tile([P, 512], f32, tag=("pe" if t4 % 2 == 0 else "po"), bufs=1)
                for co in range(NCO):
                    nc.tensor.matmul(
                        pv[:],
                        xtb[:, co, t4 * P : (t4 + 1) * P],
                        wv_sb[:, co, :],
                        start=(co == 0),
                        stop=(co == NCO - 1),
                    )
                src = pv[:].rearrange("p (hp par c) -> p hp par c", par=2, c=64)
                nc.vector.tensor_copy(
                    out=v5[:, kc, :, 0, 0:64], in_=src[:, :, 0, :]
                )
                nc.vector.tensor_copy(
                    out=v5[:, kc, :, 1, 64:128], in_=src[:, :, 1, :]
                )

            # ---- attention for this q block ----
            for hp in range(4):
                # deferred outproj blocks + next block's first proj
                # m-groups fill the PE's exp-wait gaps (attention is
                # ACT-bound); each insertion kept near the ~2-chunk
                # exp-pipeline depth so the ACT stream never runs dry
                if hp >= 2:
                    if jq < NJQ - 1:
                        get_xtb(jq + 1)
                        emit_mgroup(jq + 1, hp - 2)
                        pulled[jq + 1] = hp - 1
                    elif hp == 3:
                        # last block: recip for hp0-2 queued on DVE now; the
                        # PE-side norm pieces land a few chunks into hp3's
                        # attention so the PE never waits on this chain
                        nc.vector.reciprocal_approx_fast(
                            out=scr_sb[0:7, jq, :], in_=sc_sb[0:7, jq, :]
                        )
                        nc.vector.tensor_copy(
                            out=scb_sb[0:7, jq, :], in_=scr_sb[0:7, jq, :]
                        )
                nk = (jq + 1) * 4
                psyE = psum.tile([P, 512], f32, tag="pe", bufs=1)
                psyO = psum.tile([P, 512], f32, tag="po", bufs=1)

                def emit_av(kc, att, off, stop):
                    ga = g2(att[:])
                    nc.tensor.matmul(
                        psyE[:, off:512], v_sb[:, kc, 2 * hp, :],
                        ga[:, 0, off:512], start=(kc == 0), stop=stop,
                    )
                    nc.tensor.matmul(
                        psyO[:, off:512], v_sb[:, kc, 2 * hp + 1, :],
                        ga[:, 1, off:512], start=(kc == 0), stop=stop,
                    )

                pend = []  # software-pipelined AV emission (lag 1)
                for kc in range(nk):
                    if kc == 5 and jq == NJQ - 1 and hp == 3:
                        for h2 in range(3):  # normalize hp0-2 mid-attention
                            emit_norm_hp(jq, h2)
                    d = kc - jq * 4
                    off = P * d if d >= 0 else 0
                    att = work.tile([P, 1024], bf16, tag="att", bufs=6)
                    ps = psum.tile([P, 1024], f32, tag="qk2", bufs=3)
                    for par, sl in ((0, slice(0, 64)), (1, slice(64, 128))):
                        nc.tensor.matmul(
                            ps[:, par * 512 + off : (par + 1) * 512],
                            qkt_sb[sl, 1, hp, kc * P : (kc + 1) * P],
                            qkt_sb[sl, 0, hp, jq * 512 + off : (jq + 1) * 512],
                            start=True,
                            stop=True,
                        )
                    nc.scalar.activation(
                        g2(att[:])[:, :, off:512], g2(ps[:])[:, :, off:512],
                        AFT.Exp, scale=0.125,
                    )
                    if d >= 0:
                        nc.vector.tensor_mul(
                            out=g2(att[:])[:, :, off : off + P],
                            in0=g2(att[:])[:, :, off : off + P],
                            in1=tri_sb[:],
                        )
                    pend.append((kc, att, off))
                    if len(pend) > 1:
                        emit_av(*pend.pop(0), stop=False)
                    if kc == 1 and hp < 2 and jq > 0:
                        # two deferred outproj blocks after the first AV:
                        # their psum WAR and DVE ob copies clear well before
                        # the next head-pair boundary
                        emit_outproj_tcn(jq - 1, 2 * hp)
                        emit_outproj_tcn(jq - 1, 2 * hp + 1)
                for i, pv in enumerate(pend):
                    emit_av(*pv, stop=(i == len(pend) - 1))

                # s rows first (they gate the deferred norm's recip chain):
                # copy to SBUF rows 0/64, then DMA-gather onto compact
                # partitions hp / 4+hp of sc (cross-partition)
                nc.vector.tensor_copy(
                    out=s2_sb[64:65, hp, :], in_=psyE[64:65, :]
                )
                nc.vector.tensor_copy(
                    out=s2_sb[0:1, hp, :], in_=psyO[0:1, :]
                )
                nc.sync.dma_start(sc_sb[hp : hp + 1, jq, :], s2_sb[0:1, hp, :])
                nc.sync.dma_start(
                    sc_sb[4 + hp : 5 + hp, jq, :], s2_sb[64:65, hp, :]
                )
                nc.vector.tensor_copy(
                    out=yt_sb[0:64, hp, blk], in_=psyE[0:64, :]
                )
                nc.vector.tensor_copy(
                    out=yt_sb[64:128, hp, blk], in_=psyO[64:128, :]
                )

        # tail: tcn 12/13 accumulate dc0-2 (hp0-2 already normalized)
        # while the DVE recip chain for hp3 completes; only the dc3
        # matmuls wait on it
        jql = NJQ - 1
        po_ab = []
        for t4 in range(2):
            tcn = jql * 4 + t4
            po2 = psum.tile([P, 1024], f32, tag="qk2", bufs=3)
            po_ab.append(po2)
            for half in range(2):
                for dc in range(3):
                    nc.tensor.matmul(
                        po2[:, half * 512 : (half + 1) * 512],
                        yt_sb[:, dc, tcn * P : (tcn + 1) * P],
                        wp_sb[:, dc, half * 512 : (half + 1) * 512],
                        start=(dc == 0), stop=False,
                    )
        emit_recip(jql)
        emit_norm_hp(jql, 3)
        for t4 in range(2):
            tcn = jql * 4 + t4
            po2 = po_ab[t4]
            for half in range(2):
                nc.tensor.matmul(
                    po2[:, half * 512 : (half + 1) * 512],
                    yt_sb[:, 3, tcn * P : (tcn + 1) * P],
                    wp_sb[:, 3, half * 512 : (half + 1) * 512],
                    start=False, stop=True,
                )
            ob = work.tile([P, C], bf16, tag="ob", bufs=2)
            nc.scalar.activation(ob[:], po2[:], AFT.Copy)
            nc.sync.dma_start(out_r[:, tcn, :], ob[:])
        emit_outproj_tcn(jql, 2, use_act=True)
        emit_outproj_tcn(jql, 3, use_act=True)

    nc.finalize()
    return nc


_NC = None


def _get_nc():
    global _NC
    if _NC is None:
        _NC = build_nc()
    return _NC


def make_in_maps(x, Wk, Wq, Wv, Wp):
    x = np.asarray(x, dtype=np.float32)
    Wk = np.asarray(Wk, dtype=np.float32)
    Wq = np.asarray(Wq, dtype=np.float32)
    Wv = np.asarray(Wv, dtype=np.float32)
    Wp = np.asarray(Wp, dtype=np.float32)
    tri1 = np.triu(np.ones((P, P), np.float32))
    tri = np.stack([tri1, tri1], axis=1).astype(ml_dtypes.bfloat16)  # [P,2,P]
    bcm = np.zeros((8, 4, P), np.float32)
    for hp in range(4):
        bcm[4 + hp, hp, 0:64] = 1.0    # even head recip -> yt rows 0:64
        bcm[hp, hp, 64:128] = 1.0      # odd head recip -> yt rows 64:128
    bcm = bcm.astype(ml_dtypes.bfloat16)

    def shuf_x(xb):  # [C,T] -> [jq, p, co, t-in-block]
        return np.ascontiguousarray(
            xb.T.reshape(NCO, P, NJQ, 512).transpose(2, 1, 0, 3)
        ).astype(ml_dtypes.bfloat16)

    def shuf_w(w):  # [C,D] -> [p, co, d]
        return np.ascontiguousarray(
            w.reshape(NCO, P, D).transpose(1, 0, 2)
        ).astype(ml_dtypes.bfloat16)

    def shuf_qk(w):  # [C,D] -> [m, p, co, dd]
        return np.ascontiguousarray(
            w.reshape(NCO, P, 4, P).transpose(2, 1, 0, 3)
        ).astype(ml_dtypes.bfloat16)

    xt_b = [shuf_x(x[b]) for b in range(B)]
    in_maps = []
    for c in range(8):
        b, g = c // 2, c % 2
        sl = slice(g * D, (g + 1) * D)
        in_maps.append({
            "xt": xt_b[b],
            "wq": shuf_qk(Wq[:, sl]),
            "wk": shuf_qk(Wk[:, sl]),
            "wv": shuf_w(Wv[:, sl]),
            "wp": np.ascontiguousarray(Wp[sl, :]).astype(ml_dtypes.bfloat16),
            "tri": tri,
            "bcm": bcm,
        })
    return in_maps


def _run(x, Wk, Wq, Wv, Wp, trace=False):
    nc = _get_nc()
    in_maps = make_in_maps(x, Wk, Wq, Wv, Wp)
    res = run_bass_kernel_spmd(nc, in_maps, core_ids=list(range(8)), trace=trace)
    parts = [res.results[c]["out"] for c in range(8)]
    out = np.stack(
        [parts[2 * b] + parts[2 * b + 1] for b in range(B)], axis=0
    ).astype(np.float32)
    return out, res


def kernel(x, Wk, Wq, Wv, Wp):
    out, _ = _run(x, Wk, Wq, Wv, Wp, trace=False)
    return out
